# revision 1
# baseline (speedup 1.0000x reference)
"""CartesianMACE rank-0 fused kernel for 8 trn2 NeuronCores.

The reference's ranks 1 and 2 never reach the output (each rank is mixed
independently and the head reads only h[0]), so only the rank-0 slices of
cw0/mw0/cw1/mw1 plus h0/msg0_r0/msg1_r0/w_pred/b_pred are needed.

Per node n (16x16 mats A=cw0[0,n], B=mw0[0,n], D=cw1[0,n], E=mw1[0,n];
16-vecs x=h0[n], m0=msg0_r0[n], m1=msg1_r0[n]):
    s[n] = colsum(D) . (A x + B m0) + colsum(E) . m1
    out  = [sum_n s[n] w_pred[0,n], sum_n s[n] w_pred[1,n]] + b_pred

Sharding: data-parallel over nodes. 50000 nodes padded to 50176 =
8 cores x 7 supertiles x 128 partitions x 7 groups. Nodes live on SBUF
partitions; the 256-element flattened matrices live on the free axis.
All compute on the vector engine; per-core (128,2) partials are summed
on host (the final all-reduce of the head).
"""

import sys
import types

for _p in ("/opt/trn_rl_repo", "/root/.axon_site/_ro/trn_rl_repo"):
    if _p not in sys.path:
        sys.path.append(_p)

import numpy as np

N, CH = 50000, 16
CORES = 8
T, S = 7, 7          # supertiles per core, groups per supertile
GP = T * S           # 49 groups of 128 nodes per core
NP = CORES * T * 128 * S  # 50176 padded nodes

_cache = {}
TRACE = False
GP_MUL2 = True  # run the B*m0 mult on GpSimd
GP_MUL1 = False  # set by test harness to capture an NTFF profile


def _split_multiwait(nc, mybir):
    """This walrus build accepts a single sync-wait per instruction, but Tile
    attaches one wait per producer proc. Split: keep the last wait on the
    instruction and hoist the rest onto fresh same-engine Drain carriers
    inserted immediately before it (engines execute their stream in-order,
    so semantics are identical)."""
    for fn in nc.m.functions:
        for bb in fn.blocks:
            insts = bb.instructions  # live list
            i = 0
            while i < len(insts):
                ins = insts[i]
                si = ins.sync_info
                if si is not None and len(si.on_wait) > 1:
                    waits = list(si.on_wait)
                    ins.sync_info = mybir.SyncInfo(
                        on_wait=waits[-1:], on_update=list(si.on_update))
                    for k, w in enumerate(waits[:-1]):
                        insts.insert(i + k, mybir.InstDrain(
                            name=f"{ins.name}_w{k}", opcode="Drain",
                            engine=ins.engine, ins=[], outs=[],
                            sync_info=mybir.SyncInfo(on_wait=[w], on_update=[]),
                        ))
                    i += len(waits) - 1
                i += 1


def _build_nc():
    import concourse.bass as bass
    import concourse.tile as tile
    import concourse.mybir as mybir

    f32 = mybir.dt.float32
    P = 128

    nc = bass.Bass("TRN2", target_bir_lowering=False, debug=False,
                   num_devices=CORES)

    ab_d = nc.dram_tensor("ab", [T, P, S * 512], f32,
                          kind="ExternalInput").ap()
    de_d = nc.dram_tensor("de", [T, P, S * 512], f32,
                          kind="ExternalInput").ap()
    xm_d = nc.dram_tensor("xm", [P, T * S * 32], f32,
                          kind="ExternalInput").ap()
    m1_d = nc.dram_tensor("m1", [P, T * S * 16], f32,
                          kind="ExternalInput").ap()
    w_d = nc.dram_tensor("w", [P, 2 * GP], f32, kind="ExternalInput").ap()
    o_d = nc.dram_tensor("o", [P, 2], f32, kind="ExternalOutput").ap()

    with tile.TileContext(nc) as tc:
        with (
            tc.tile_pool(name="mats", bufs=4) as mats,
            tc.tile_pool(name="vecs", bufs=3) as vecs,
            tc.tile_pool(name="work", bufs=2) as work,
            tc.tile_pool(name="acc", bufs=1) as acc,
        ):
            # persistent accumulators, finalized after the loop
            tvm_all = acc.tile([P, 2 * GP * 16], f32)   # [tA | tB] row sums
            deq_all = acc.tile([P, 2 * GP * 16], f32)   # [d | e] colsums
            v_all = acc.tile([P, 2 * GP * 16], f32)     # [tv | m1]
            w_sb = acc.tile([P, 2 * GP], f32)
            xm_all = acc.tile([P, T * S * 32], f32)
            nc.sync.dma_start(out=xm_all[:, :], in_=xm_d)

            for t in range(T):
                ab_sb = mats.tile([P, S * 512], f32, tag="ab")
                nc.sync.dma_start(out=ab_sb[:, 0:S * 256],
                                  in_=ab_d[t][:, 0:S * 256])
                nc.sync.dma_start(out=ab_sb[:, S * 256:S * 512],
                                  in_=ab_d[t][:, S * 256:S * 512])
                de_sb = mats.tile([P, S * 512], f32, tag="de")
                nc.sync.dma_start(out=de_sb[:, :], in_=de_d[t])
                xm_sb = xm_all[:, t * S * 32:(t + 1) * S * 32]

                # tmp[m,g,j,k] = {A,B}[g,j,k] * {x,m0}[g,k]
                gjk = lambda ap: ap.rearrange("p (g j k) -> p g j k",
                                              g=S, j=16, k=16)
                bc = lambda ap: (ap.rearrange("p (g k) -> p g k", g=S, k=16)
                                 .unsqueeze(2).broadcast_to((P, S, 16, 16)))
                H = S * 256
                tmp = work.tile([P, S * 512], f32, tag="tmp")
                tmp5 = tmp[:, :].rearrange("p (m g j k) -> p m g j k",
                                           m=2, g=S, j=16, k=16)
                MUL1 = nc.gpsimd if GP_MUL1 else nc.vector
                MUL1.tensor_mul(out=gjk(tmp[:, 0:H]),
                                in0=gjk(ab_sb[:, 0:H]),
                                in1=bc(xm_sb[:, 0:S * 16]))
                MUL2 = nc.gpsimd if GP_MUL2 else nc.vector
                MUL2.tensor_mul(out=gjk(tmp[:, H:2 * H]),
                                in0=gjk(ab_sb[:, H:2 * H]),
                                in1=bc(xm_sb[:, S * 16:S * 32]))

                # row sums into tvm_all[:, m, t, g, j]
                nc.vector.reduce_sum(
                    out=tvm_all[:, :].rearrange("p (m t g j) -> p m t g j",
                                                m=2, t=T, g=S, j=16)[:, :, t],
                    in_=tmp5, axis=mybir.AxisListType.X)

                # colsums: D,E host-transposed (k-major), j contiguous.
                # GpSimd folds j 16->8, DVE reduces the remaining 8.
                h8 = work.tile([P, S * 256], f32, tag="h8")
                de4 = de_sb[:, :].rearrange("p (q k j) -> p q k j",
                                            q=2 * S, k=16, j=16)
                nc.gpsimd.tensor_add(
                    out=h8[:, :].rearrange("p (q k j) -> p q k j",
                                           q=2 * S, k=16, j=8),
                    in0=de4[:, :, :, 0:8], in1=de4[:, :, :, 8:16])
                nc.vector.reduce_sum(
                    out=deq_all[:, :].rearrange("p (m t g k) -> p m t g k",
                                                m=2, t=T, g=S, k=16)[:, :, t],
                    in_=h8[:, :].rearrange("p (m g k j) -> p m g k j",
                                           m=2, g=S, k=16, j=8),
                    axis=mybir.AxisListType.X)

            # ---- epilogue: all the small per-group math, once ----
            nc.sync.dma_start(out=v_all[:, GP * 16:2 * GP * 16], in_=m1_d)
            nc.sync.dma_start(out=w_sb[:, :], in_=w_d)
            nc.vector.tensor_add(out=v_all[:, 0:GP * 16],
                                 in0=tvm_all[:, 0:GP * 16],
                                 in1=tvm_all[:, GP * 16:2 * GP * 16])
            pr = acc.tile([P, 2 * GP * 16], f32)
            nc.vector.tensor_mul(out=pr[:, :], in0=deq_all[:, :],
                                 in1=v_all[:, :])
            sm = acc.tile([P, 2 * GP], f32)
            nc.vector.reduce_sum(
                out=sm[:, :].rearrange("p (m tg) -> p m tg", m=2, tg=GP),
                in_=pr[:, :].rearrange("p (m tg k) -> p m tg k",
                                       m=2, tg=GP, k=16),
                axis=mybir.AxisListType.X)
            s_all = acc.tile([P, GP], f32)
            nc.vector.tensor_add(out=s_all[:, :], in0=sm[:, 0:GP],
                                 in1=sm[:, GP:2 * GP])
            # head: o[:, c] = sum_g s_all[:, g] * w[:, c*GP+g]
            junk = acc.tile([P, 2 * GP], f32)
            nc.vector.tensor_mul(
                out=junk[:, :].rearrange("p (c g) -> p c g", c=2, g=GP),
                in0=s_all[:, :].rearrange("p g -> p g").unsqueeze(1)
                .broadcast_to((P, 2, GP)),
                in1=w_sb[:, :].rearrange("p (c g) -> p c g", c=2, g=GP))
            o_sb = acc.tile([P, 2], f32)
            nc.vector.reduce_sum(
                out=o_sb[:, :].rearrange("p c -> p c"),
                in_=junk[:, :].rearrange("p (c g) -> p c g", c=2, g=GP),
                axis=mybir.AxisListType.X)
            nc.sync.dma_start(out=o_d, in_=o_sb[:, :])

    return nc


def _get_nc():
    if "nc" not in _cache:
        _cache["nc"] = _build_nc()
    return _cache["nc"]


def _shard_mat(m):
    """(N,16,16) -> (CORES, T, 128, S*256), zero-padded, group-major free axis."""
    out = np.zeros((NP, 256), np.float32)
    out[:N] = np.asarray(m, np.float32).reshape(N, 256)
    return np.ascontiguousarray(out.reshape(CORES, T, 128, S * 256))


def _shard_vec(v):
    """(N,16) -> (CORES, T, 128, S*16)."""
    out = np.zeros((NP, 16), np.float32)
    out[:N] = np.asarray(v, np.float32).reshape(N, 16)
    return np.ascontiguousarray(out.reshape(CORES, T, 128, S * 16))


def kernel(h0, cw0, mw0, cw1, mw1,
           msg0_r0, msg0_r1, msg0_r2,
           msg1_r0, msg1_r1, msg1_r2,
           w_pred, b_pred):
    from concourse.bass_utils import run_bass_kernel_spmd

    nc = _get_nc()
    if not _cache.get("split_done"):
        import concourse.mybir as mybir
        _split_multiwait(nc, mybir)
        _cache["split_done"] = True

    A4 = _shard_mat(cw0[0]).reshape(CORES, T, 128, S, 256)
    B4 = _shard_mat(mw0[0]).reshape(CORES, T, 128, S, 256)
    AB = np.ascontiguousarray(
        np.stack([A4, B4], axis=3).reshape(CORES, T, 128, S * 512))
    DE = np.ascontiguousarray(
        np.stack([_shard_mat(np.swapaxes(np.asarray(cw1[0], np.float32), 1, 2))
                  .reshape(CORES, T, 128, S, 256),
                  _shard_mat(np.swapaxes(np.asarray(mw1[0], np.float32), 1, 2))
                  .reshape(CORES, T, 128, S, 256)],
                 axis=3).reshape(CORES, T, 128, S * 512))
    X = _shard_vec(np.asarray(h0, np.float32)[..., 0])
    M0 = _shard_vec(np.asarray(msg0_r0, np.float32)[..., 0])
    XM = np.ascontiguousarray(
        np.stack([X, M0], axis=3).reshape(CORES, T, 128, S * 32)
        .transpose(0, 2, 1, 3).reshape(CORES, 128, T * S * 32))
    M1 = np.ascontiguousarray(
        _shard_vec(np.asarray(msg1_r0, np.float32)[..., 0])
        .transpose(0, 2, 1, 3).reshape(CORES, 128, T * S * 16))

    wp = np.zeros((2, NP), np.float32)
    wp[:, :N] = np.asarray(w_pred, np.float32)
    # (2, CORES, T, 128, S) -> (CORES, 128, 2, T, S) -> (CORES, 128, 2*GP)
    W = np.ascontiguousarray(
        wp.reshape(2, CORES, T, 128, S).transpose(1, 3, 0, 2, 4)
        .reshape(CORES, 128, 2 * GP))

    in_maps = [
        {"ab": AB[i], "de": DE[i], "xm": XM[i], "m1": M1[i], "w": W[i]}
        for i in range(CORES)
    ]
    res = run_bass_kernel_spmd(nc, in_maps, list(range(CORES)), trace=TRACE)
    _cache["last_res"] = res
    partial = np.zeros(2, np.float64)
    for i in range(CORES):
        partial += res.results[i]["o"].astype(np.float64).sum(axis=0)
    out = (partial + np.asarray(b_pred, np.float64)).astype(np.float32)
    return out.reshape(1, 2)



# revision 7
# speedup vs baseline: 1.1038x; 1.1038x over previous
"""CartesianMACE rank-0 fused kernel for 8 trn2 NeuronCores (v3).

Only the rank-0 path reaches the output (ranks 1/2 of the reference are
dead code), so per node n with 16x16 mats A=cw0[0,n], B=mw0[0,n],
D=cw1[0,n], E=mw1[0,n] and 16-vecs x=h0[n], m0=msg0_r0[n], m1=msg1_r0[n]:

    s[n] = colsum(D) . (A x + B m0) + colsum(E) . m1
    out  = [sum_n s[n] w_pred[0,n], sum_n s[n] w_pred[1,n]] + b_pred

v3 design (vs the f32 baseline at ~98us):
  * All streamed data is bf16 -> halves HBM traffic (13.4MB/core) and
    enables the DVE 2x perf mode for tensor_tensor ops.
  * Row-sums of A*x / B*m0 are bf16 fold-trees (16->8->4->2->1), not 1x
    reduce_sum; computed over 2-supertile spans to amortize per-op cost.
  * colsum(D)/colsum(E) are computed BY THE DMA ENGINES: a SWDGE
    (gpsimd) dma with accum_op=add streams the 16 j-slices of D^T/E^T
    from HBM onto one SBUF tile (stride-0 output AP), so no compute
    engine touches the 6.4MB of D/E data. Fallback MODE="v2" does the
    first fold on GpSimd and the rest on DVE.
  * Nodes on SBUF partitions: 50000 padded to 50176 = 8 cores x 7
    supertiles x 128 partitions x 7 groups. Per-core (128,2) partials
    are summed on host with b_pred (the head's all-reduce).
"""

import sys

for _p in ("/opt/trn_rl_repo", "/root/.axon_site/_ro/trn_rl_repo"):
    if _p not in sys.path:
        sys.path.append(_p)

import numpy as np
import ml_dtypes

BF16 = ml_dtypes.bfloat16

N, CH = 50000, 16
CORES = 8
T, S = 7, 7          # supertiles per core, groups per supertile
GP = T * S           # 49 groups of 128 nodes per core
NP = CORES * T * 128 * S  # 50176 padded nodes
SPANS = [(0, 2), (2, 2), (4, 2), (6, 1)]  # compute spans over supertiles

_cache = {}
TRACE = False
MODE = "v2"  # "v3": DE colsum via DMA-accum; "v2": GpSimd fold1 + DVE folds


def _split_multiwait(nc, mybir):
    """This walrus build accepts a single sync-wait per instruction, but Tile
    attaches one wait per producer proc. Split: keep the last wait on the
    instruction and hoist the rest onto fresh same-engine Drain carriers
    inserted immediately before it (engines execute their stream in-order,
    so semantics are identical)."""
    for fn in nc.m.functions:
        for bb in fn.blocks:
            insts = bb.instructions  # live list
            i = 0
            while i < len(insts):
                ins = insts[i]
                si = ins.sync_info
                if si is not None and len(si.on_wait) > 1:
                    waits = list(si.on_wait)
                    ins.sync_info = mybir.SyncInfo(
                        on_wait=waits[-1:], on_update=list(si.on_update))
                    for k, w in enumerate(waits[:-1]):
                        insts.insert(i + k, mybir.InstDrain(
                            name=f"{ins.name}_w{k}", opcode="Drain",
                            engine=ins.engine, ins=[], outs=[],
                            sync_info=mybir.SyncInfo(on_wait=[w], on_update=[]),
                        ))
                    i += len(waits) - 1
                i += 1


def _build_nc():
    import concourse.bass as bass
    import concourse.tile as tile
    import concourse.mybir as mybir

    f32 = mybir.dt.float32
    b16 = mybir.dt.bfloat16
    P = 128

    nc = bass.Bass("TRN2", target_bir_lowering=False, debug=False,
                   num_devices=CORES)

    ab_d = nc.dram_tensor("ab", [T, P, S * 512], b16,
                          kind="ExternalInput").ap()
    if MODE == "v3":
        # [p, j, (t, m, g, k)] -- j-major slices for the accumulating DMA
        de_d = nc.dram_tensor("de", [P, 16, T * 2 * S * 16], b16,
                              kind="ExternalInput").ap()
    else:
        # [t][p, (m, g, k, j)] -- j innermost for fold-trees
        de_d = nc.dram_tensor("de", [T, P, S * 512], b16,
                              kind="ExternalInput").ap()
    xm_d = nc.dram_tensor("xm", [P, T * S * 32], b16,
                          kind="ExternalInput").ap()
    m1_d = nc.dram_tensor("m1", [P, T * S * 16], b16,
                          kind="ExternalInput").ap()
    w_d = nc.dram_tensor("w", [P, 2 * GP], b16, kind="ExternalInput").ap()
    o_d = nc.dram_tensor("o", [P, 2], f32, kind="ExternalOutput").ap()

    FREE = T * 2 * S * 16  # 1568: [t, m, g, k] per-partition layout

    with tile.TileContext(nc) as tc:
        with (
            tc.tile_pool(name="big", bufs=1) as big,
            tc.tile_pool(name="work", bufs=2) as work,
        ):
            ab_all = big.tile([P, T * S * 512], b16)   # all AB data
            xm_sb = big.tile([P, T * S * 32], b16)
            m1_sb = big.tile([P, T * S * 16], b16)
            w_sb = big.tile([P, 2 * GP], b16)
            cd_all = big.tile([P, FREE], b16)          # [t,m,g,k] colsums
            t1_all = big.tile([P, FREE], b16)          # [t,m,g,j] row sums
            vv = big.tile([P, FREE], b16)              # [t, (t|m1), g, k]
            pr = big.tile([P, FREE], b16)

            # small inputs first on the HWDGE queue
            nc.sync.dma_start(out=xm_sb[:, :], in_=xm_d)
            nc.sync.dma_start(out=m1_sb[:, :], in_=m1_d)
            nc.sync.dma_start(out=w_sb[:, :], in_=w_d)

            if MODE == "v3":
                nc.vector.memset(cd_all[:, :], 0.0)
                # colsum over j happens inside the DMA engines (CCE add):
                # out AP revisits cd_all once per j-slice.
                cd_bc = cd_all[:, :].unsqueeze(1).broadcast_to((P, 16, FREE))
                nc.gpsimd.dma_start(out=cd_bc, in_=de_d,
                                    accum_op=mybir.AluOpType.add)
            else:
                de_all = big.tile([P, T * S * 512], b16)
                d8_all = big.tile([P, T * S * 256], b16)

            for t in range(T):
                nc.sync.dma_start(
                    out=ab_all[:, t * 3584:(t + 1) * 3584], in_=ab_d[t])
                if MODE == "v2":
                    nc.sync.dma_start(
                        out=de_all[:, t * 3584:(t + 1) * 3584], in_=de_d[t])

            for (t0, ts) in SPANS:
                U = ts * 2 * S          # (t, m, g) groups in span
                E0 = t0 * 3584          # elem offset into ab_all
                EN = ts * 3584
                V0 = t0 * 224           # elem offset into t1_all/cd_all
                ab_sp = ab_all[:, E0:E0 + EN]
                gjk = lambda ap: ap.rearrange("p (u j k) -> p u j k",
                                              u=U, j=16, k=16)
                xm_bc = (xm_sb[:, t0 * 224:(t0 + ts) * 224]
                         .rearrange("p (u k) -> p u k", u=U, k=16)
                         .unsqueeze(2).broadcast_to((P, U, 16, 16)))
                tmp = work.tile([P, 2 * 3584], b16, tag="tmp")
                t8 = work.tile([P, 2 * 1792], b16, tag="t8")
                t4 = work.tile([P, 2 * 1792], b16, tag="t4")
                t2 = work.tile([P, 2 * 896], b16, tag="t2")
                tm = tmp[:, :EN].rearrange("p (v k) -> p v k", v=U * 16, k=16)
                nc.vector.tensor_mul(out=gjk(tmp[:, :EN]), in0=gjk(ab_sp),
                                     in1=xm_bc)
                e8 = t8[:, :U * 128].rearrange("p (v k) -> p v k",
                                               v=U * 16, k=8)
                nc.vector.tensor_add(out=e8, in0=tm[:, :, 0:8],
                                     in1=tm[:, :, 8:16])
                e4 = t4[:, :U * 64].rearrange("p (v k) -> p v k",
                                              v=U * 16, k=4)
                nc.vector.tensor_add(out=e4, in0=e8[:, :, 0:4],
                                     in1=e8[:, :, 4:8])
                e2 = t2[:, :U * 32].rearrange("p (v k) -> p v k",
                                              v=U * 16, k=2)
                nc.vector.tensor_add(out=e2, in0=e4[:, :, 0:2],
                                     in1=e4[:, :, 2:4])
                nc.vector.tensor_add(
                    out=t1_all[:, V0:(t0 + ts) * 224],
                    in0=e2[:, :, 0], in1=e2[:, :, 1])

                if MODE == "v2":
                    de_sp = de_all[:, E0:E0 + EN].rearrange(
                        "p (v j) -> p v j", v=U * 16, j=16)
                    d8 = d8_all[:, E0 // 2:(E0 + EN) // 2].rearrange(
                        "p (v j) -> p v j", v=U * 16, j=8)
                    nc.gpsimd.tensor_add(out=d8, in0=de_sp[:, :, 0:8],
                                         in1=de_sp[:, :, 8:16])
                    d4 = t4[:, U * 64:U * 128].rearrange(
                        "p (v j) -> p v j", v=U * 16, j=4)
                    nc.vector.tensor_add(out=d4, in0=d8[:, :, 0:4],
                                         in1=d8[:, :, 4:8])
                    d2 = t2[:, U * 32:U * 64].rearrange(
                        "p (v j) -> p v j", v=U * 16, j=2)
                    nc.vector.tensor_add(out=d2, in0=d4[:, :, 0:2],
                                         in1=d4[:, :, 2:4])
                    nc.vector.tensor_add(
                        out=cd_all[:, V0:(t0 + ts) * 224],
                        in0=d2[:, :, 0], in1=d2[:, :, 1])

            # ---- epilogue ----
            tmg = lambda ap: ap.rearrange("p (t m x) -> p t m x",
                                          t=T, m=2, x=S * 16)
            # vv[t,0,g,k] = t1[t,0,g,j] + t1[t,1,g,j];  vv[t,1,g,k] = m1
            nc.vector.tensor_add(out=tmg(vv[:, :])[:, :, 0],
                                 in0=tmg(t1_all[:, :])[:, :, 0],
                                 in1=tmg(t1_all[:, :])[:, :, 1])
            nc.vector.tensor_copy(
                tmg(vv[:, :])[:, :, 1],
                m1_sb[:, :].rearrange("p (t x) -> p t x", t=T, x=S * 16))
            nc.vector.tensor_mul(out=pr[:, :], in0=cd_all[:, :],
                                 in1=vv[:, :])
            g8 = t1_all[:, 0:784].rearrange("p (v k) -> p v k", v=98, k=8)
            p16 = pr[:, :].rearrange("p (v k) -> p v k", v=98, k=16)
            nc.vector.tensor_add(out=g8, in0=p16[:, :, 0:8],
                                 in1=p16[:, :, 8:16])
            g4 = t1_all[:, 784:1176].rearrange("p (v k) -> p v k", v=98, k=4)
            nc.vector.tensor_add(out=g4, in0=g8[:, :, 0:4], in1=g8[:, :, 4:8])
            g2 = t1_all[:, 1176:1372].rearrange("p (v k) -> p v k", v=98, k=2)
            nc.vector.tensor_add(out=g2, in0=g4[:, :, 0:2], in1=g4[:, :, 2:4])
            g1 = t1_all[:, 1372:1470]
            nc.vector.tensor_add(out=g1, in0=g2[:, :, 0], in1=g2[:, :, 1])
            # s[t,g] = g1[t,0,g] + g1[t,1,g]
            s_all = t1_all[:, 1470:1519]
            gm = g1.rearrange("p (t m g) -> p t m g", t=T, m=2, g=S)
            nc.vector.tensor_add(out=s_all.rearrange("p (t g) -> p t g",
                                                     t=T, g=S),
                                 in0=gm[:, :, 0], in1=gm[:, :, 1])
            # head: hm[c, tg] = s[tg] * w[c, tg]; o[c] = sum_tg hm
            hm = pr[:, 0:2 * GP].rearrange("p (c q) -> p c q", c=2, q=GP)
            nc.vector.tensor_mul(
                out=hm,
                in0=w_sb[:, :].rearrange("p (c q) -> p c q", c=2, q=GP),
                in1=s_all.unsqueeze(1).broadcast_to((P, 2, GP)))
            o_sb = big.tile([P, 2], f32)
            nc.vector.reduce_sum(out=o_sb[:, :].rearrange("p c -> p c"),
                                 in_=hm, axis=mybir.AxisListType.X)
            nc.sync.dma_start(out=o_d, in_=o_sb[:, :])

    return nc


def _get_nc():
    if "nc" not in _cache:
        _cache["nc"] = _build_nc()
        import concourse.mybir as mybir
        _split_multiwait(_cache["nc"], mybir)
    return _cache["nc"]


def kernel(h0, cw0, mw0, cw1, mw1,
           msg0_r0, msg0_r1, msg0_r2,
           msg1_r0, msg1_r1, msg1_r2,
           w_pred, b_pred):
    from concourse.bass_utils import run_bass_kernel_spmd

    nc = _get_nc()

    def pad_mat(m):
        out = np.zeros((NP, 256), np.float32)
        out[:N] = np.asarray(m, np.float32).reshape(N, 256)
        return out.reshape(CORES, T, 128, S, 16, 16)  # [c,t,p,g,j,k]

    A5 = pad_mat(cw0[0])
    B5 = pad_mat(mw0[0])
    # AB: [c,t,p, m,g,j,k] -> (c,t,p,3584)
    AB = np.ascontiguousarray(
        np.stack([A5, B5], axis=3).reshape(CORES, T, 128, S * 512)
    ).astype(BF16)

    D5 = pad_mat(cw1[0])
    E5 = pad_mat(mw1[0])
    if MODE == "v3":
        # [c,t,p,g,j,k] -> [c, p, j, t, m, g, k] -> (c, p, 16, 1568)
        DE = np.ascontiguousarray(
            np.stack([D5, E5], axis=3)        # c t p m g j k
            .transpose(0, 2, 5, 1, 3, 4, 6)   # c p j t m g k
            .reshape(CORES, 128, 16, T * 2 * S * 16)).astype(BF16)
    else:
        # j innermost: [c,t,p, m,g,k,j] -> (c,t,p,3584)
        DE = np.ascontiguousarray(
            np.stack([D5.transpose(0, 1, 2, 3, 5, 4),
                      E5.transpose(0, 1, 2, 3, 5, 4)], axis=3)
            .reshape(CORES, T, 128, S * 512)).astype(BF16)

    def pad_vec(v):
        out = np.zeros((NP, 16), np.float32)
        out[:N] = np.asarray(v, np.float32).reshape(N, 16)
        return out.reshape(CORES, T, 128, S, 16)

    X = pad_vec(np.asarray(h0, np.float32)[..., 0])
    M0 = pad_vec(np.asarray(msg0_r0, np.float32)[..., 0])
    XM = np.ascontiguousarray(
        np.stack([X, M0], axis=3).reshape(CORES, T, 128, S * 32)
        .transpose(0, 2, 1, 3).reshape(CORES, 128, T * S * 32)).astype(BF16)
    M1 = np.ascontiguousarray(
        pad_vec(np.asarray(msg1_r0, np.float32)[..., 0])
        .reshape(CORES, T, 128, S * 16)
        .transpose(0, 2, 1, 3).reshape(CORES, 128, T * S * 16)).astype(BF16)

    wp = np.zeros((2, NP), np.float32)
    wp[:, :N] = np.asarray(w_pred, np.float32)
    W = np.ascontiguousarray(
        wp.reshape(2, CORES, T, 128, S).transpose(1, 3, 0, 2, 4)
        .reshape(CORES, 128, 2 * GP)).astype(BF16)

    in_maps = [
        {"ab": AB[i], "de": DE[i], "xm": XM[i], "m1": M1[i], "w": W[i]}
        for i in range(CORES)
    ]
    res = run_bass_kernel_spmd(nc, in_maps, list(range(CORES)), trace=TRACE)
    _cache["last_res"] = res
    partial = np.zeros(2, np.float64)
    for i in range(CORES):
        partial += res.results[i]["o"].astype(np.float64).sum(axis=0)
    out = (partial + np.asarray(b_pred, np.float64)).astype(np.float32)
    return out.reshape(1, 2)


# revision 13
# speedup vs baseline: 1.3711x; 1.2422x over previous
"""CartesianMACE rank-0 fused kernel for 8 trn2 NeuronCores (v5).

Only the rank-0 path reaches the output (ranks 1/2 of the reference are
dead code), so per node n with 16x16 mats A=cw0[0,n], B=mw0[0,n],
D=cw1[0,n], E=mw1[0,n] and 16-vecs x=h0[n], m0=msg0_r0[n], m1=msg1_r0[n]:

    s[n] = colsum(D) . (A x + B m0) + colsum(E) . m1
    out  = [sum_n s[n] w_pred[0,n], sum_n s[n] w_pred[1,n]] + b_pred

Design (f32 baseline ~98us; v4 ~71us):
  * All streamed data bf16: halves HBM traffic (13.4MB/core) and gives
    DVE tensor_tensor the 2x perf mode (hardware-verified in traces).
  * DVE-ONLY compute. GpSimd and DVE arbitrate an exclusive lock on the
    shared SBUF port pair: a running GpSimd tensor op makes concurrent
    bf16 2x DVE ops 2.5-4x slower (measured), so GpSimd is kept idle.
  * Reductions are bf16 pairwise fold-trees (2x mode), not 1x
    reduce_sum. A-side products and D/E tiles fold into one shared t8
    tile so deeper levels cover both trees in single fat instructions.
  * The last fold level (stride-2 operands -> 1x mode) is skipped:
    pairs survive into the epilogue, where the cd*t dot product runs
    at doubled width in 2x mode instead (net win).
  * de lands in its own per-span tiles - sharing a tile between DMA
    writes and the mul's engine writes creates a false WAW dependency
    that stalls the ramp (cost ~4us in v4).
  * Nodes on SBUF partitions: 50000 padded to 50176 = 8 cores x 7
    supertiles x 128 partitions x 7 groups. Spans [1,2,2,2] supertiles;
    all DMAs HWDGE, issued upfront, interleaved ab/de. Per-core (128,2)
    partials are summed on host with b_pred (the head's all-reduce).
"""

import sys

for _p in ("/opt/trn_rl_repo", "/root/.axon_site/_ro/trn_rl_repo"):
    if _p not in sys.path:
        sys.path.append(_p)

import numpy as np
import ml_dtypes

BF16 = ml_dtypes.bfloat16

N, CH = 50000, 16
CORES = 8
T, S = 7, 7          # supertiles per core, groups per supertile
GP = T * S           # 49 groups of 128 nodes per core
NP = CORES * T * 128 * S  # 50176 padded nodes
SPANS = [(0, 1), (1, 2), (3, 2), (5, 2)]  # (first supertile, length)

_cache = {}
TRACE = False


def _split_multiwait(nc, mybir):
    """This walrus build accepts a single sync-wait per instruction, but Tile
    attaches one wait per producer proc. Split: keep the last wait on the
    instruction and hoist the rest onto fresh same-engine Drain carriers
    inserted immediately before it (engines execute their stream in-order,
    so semantics are identical)."""
    for fn in nc.m.functions:
        for bb in fn.blocks:
            insts = bb.instructions  # live list
            i = 0
            while i < len(insts):
                ins = insts[i]
                si = ins.sync_info
                if si is not None and len(si.on_wait) > 1:
                    waits = list(si.on_wait)
                    ins.sync_info = mybir.SyncInfo(
                        on_wait=waits[-1:], on_update=list(si.on_update))
                    for k, w in enumerate(waits[:-1]):
                        insts.insert(i + k, mybir.InstDrain(
                            name=f"{ins.name}_w{k}", opcode="Drain",
                            engine=ins.engine, ins=[], outs=[],
                            sync_info=mybir.SyncInfo(on_wait=[w], on_update=[]),
                        ))
                    i += len(waits) - 1
                i += 1


def _build_nc():
    import concourse.bass as bass
    import concourse.tile as tile
    import concourse.mybir as mybir

    f32 = mybir.dt.float32
    b16 = mybir.dt.bfloat16
    P = 128

    nc = bass.Bass("TRN2", target_bir_lowering=False, debug=False,
                   num_devices=CORES)

    ab_d = nc.dram_tensor("ab", [T, P, 3584], b16, kind="ExternalInput").ap()
    de_d = nc.dram_tensor("de", [T, P, 3584], b16, kind="ExternalInput").ap()
    xm_d = nc.dram_tensor("xm", [P, T * 224], b16, kind="ExternalInput").ap()
    m1_d = nc.dram_tensor("m1", [P, T * 112], b16, kind="ExternalInput").ap()
    w_d = nc.dram_tensor("w", [P, 2 * GP], b16, kind="ExternalInput").ap()
    o_d = nc.dram_tensor("o", [P, 2], f32, kind="ExternalOutput").ap()

    F2R = 2 * T * 224  # 3136: [t, m, g, k, r2] per-partition layout
    F1 = T * 224       # 1568

    with tile.TileContext(nc) as tc:
        with (
            tc.tile_pool(name="big", bufs=1) as big,
            tc.tile_pool(name="work", bufs=1) as work,
        ):
            ab_all = big.tile([P, T * 3584], b16)
            xm_sb = big.tile([P, T * 224], b16)
            w_sb = big.tile([P, 2 * GP], b16)
            # ct2[:, 0:3136] = cd pairs (D|E colsums), [:, 3136:] = t pairs
            ct2 = big.tile([P, 2 * F2R], b16)
            vv = big.tile([P, F1], b16)     # [t, sel, g, k]: tn | m1
            cdf = big.tile([P, F1], b16)    # [t, m, g, k]: cd | ce
            pr = big.tile([P, F1], b16)
            tn2 = big.tile([P, F1], b16)    # m-summed t pairs

            nc.sync.dma_start(out=xm_sb[:, :], in_=xm_d)
            # m1 lands directly in its epilogue slot (sel=1 planes of vv)
            nc.sync.dma_start(
                out=vv[:, :].rearrange("p (t s x) -> p t s x",
                                       t=T, s=2, x=112)[:, :, 1],
                in_=m1_d.rearrange("p (t x) -> p t x", t=T, x=112))
            des = []
            for (t0, ts) in SPANS:
                for u in range(ts):
                    t = t0 + u
                    nc.sync.dma_start(
                        out=ab_all[:, t * 3584:(t + 1) * 3584], in_=ab_d[t])
                de = work.tile([P, 2 * 3584], b16, tag="de", bufs=3)
                des.append(de)
                for u in range(ts):
                    nc.sync.dma_start(out=de[:, u * 3584:(u + 1) * 3584],
                                      in_=de_d[t0 + u])
            nc.sync.dma_start(out=w_sb[:, :], in_=w_d)

            t4 = work.tile([P, 2 * 1792], b16)
            for si, (t0, ts) in enumerate(SPANS):
                de = des[si]
                EN = ts * 3584
                U = ts * 14          # (t, m, g) groups in span
                tmp = work.tile([P, 2 * 3584], b16, tag="tmp", bufs=2)
                t8 = work.tile([P, 2 * 3584], b16, tag="t8", bufs=2)
                gjk = lambda ap: ap.rearrange("p (u j k) -> p u j k",
                                              u=U, j=16, k=16)
                xm_bc = (xm_sb[:, t0 * 224:(t0 + ts) * 224]
                         .rearrange("p (u k) -> p u k", u=U, k=16)
                         .unsqueeze(2).broadcast_to((P, U, 16, 16)))
                nc.vector.tensor_mul(out=gjk(tmp[:, 0:EN]),
                                     in0=gjk(ab_all[:, t0 * 3584:
                                                    (t0 + ts) * 3584]),
                                     in1=xm_bc)
                # fold 16->8: de half and tmp half -> adjacent halves of t8
                HV = ts * 224        # 16-wide rows per half
                d16 = de[:, 0:EN].rearrange("p (v k) -> p v k", v=HV, k=16)
                a16 = tmp[:, 0:EN].rearrange("p (v k) -> p v k", v=HV, k=16)
                e8d = t8[:, 0:HV * 8].rearrange("p (v k) -> p v k",
                                                v=HV, k=8)
                e8a = t8[:, HV * 8:HV * 16].rearrange("p (v k) -> p v k",
                                                      v=HV, k=8)
                nc.vector.tensor_add(out=e8d, in0=d16[:, :, 0:8],
                                     in1=d16[:, :, 8:16])
                nc.vector.tensor_add(out=e8a, in0=a16[:, :, 0:8],
                                     in1=a16[:, :, 8:16])
                # merged 8->4 over [de | tmp]
                V = ts * 448
                e8 = t8[:, 0:V * 8].rearrange("p (v k) -> p v k", v=V, k=8)
                e4 = t4[:, 0:V * 4].rearrange("p (v k) -> p v k", v=V, k=4)
                nc.vector.tensor_add(out=e4, in0=e8[:, :, 0:4],
                                     in1=e8[:, :, 4:8])
                # merged 4->2, pairs kept: -> two segments of ct2
                ct_v = (ct2[:, :].rearrange("p (c f) -> p c f", c=2, f=F2R)
                        [:, :, t0 * 448:(t0 + ts) * 448]
                        .rearrange("p c (y r) -> p c y r",
                                   y=ts * 224, r=2))
                nc.vector.tensor_add(
                    out=ct_v,
                    in0=e4[:, :, 0:2].rearrange("p (c y) r -> p c y r",
                                                c=2, y=ts * 224),
                    in1=e4[:, :, 2:4].rearrange("p (c y) r -> p c y r",
                                                c=2, y=ts * 224))

            # ---- epilogue ----
            cd2 = ct2[:, 0:F2R]                  # [t, m, g, k, r] pairs
            t12 = ct2[:, F2R:2 * F2R]
            tmx = lambda ap: ap.rearrange("p (t m x) -> p t m x",
                                          t=T, m=2, x=224)
            # tn2[t,g,k,r] = t12[t,0,..] + t12[t,1,..]   (m-sum, 2x)
            tn2h = tn2[:, 0:F1].rearrange("p (t x) -> p t x", t=T, x=224)
            nc.vector.tensor_add(out=tn2h, in0=tmx(t12)[:, :, 0],
                                 in1=tmx(t12)[:, :, 1])
            # collapse pairs (1x, small): tn -> vv[sel=0]; cd2 -> cdf
            tr = tn2[:, 0:F1].rearrange("p (v r) -> p v r", v=F1 // 2, r=2)
            nc.vector.tensor_add(
                out=vv[:, :].rearrange("p (t s x) -> p t s x",
                                       t=T, s=2, x=112)[:, :, 0],
                in0=tr[:, :, 0].rearrange("p (t x) -> p t x", t=T, x=112),
                in1=tr[:, :, 1].rearrange("p (t x) -> p t x", t=T, x=112))
            cr = cd2.rearrange("p (v r) -> p v r", v=F2R // 2, r=2)
            nc.vector.tensor_add(out=cdf[:, :], in0=cr[:, :, 0],
                                 in1=cr[:, :, 1])
            # pr[t,m,g,k] = cdf * (tn | m1)
            nc.vector.tensor_mul(out=pr[:, :], in0=cdf[:, :], in1=vv[:, :])
            # fold [98, 16] -> [98]
            p16 = pr[:, :].rearrange("p (v k) -> p v k", v=98, k=16)
            h8 = tn2[:, 0:784].rearrange("p (v k) -> p v k", v=98, k=8)
            nc.vector.tensor_add(out=h8, in0=p16[:, :, 0:8],
                                 in1=p16[:, :, 8:16])
            h4 = tn2[:, 784:1176].rearrange("p (v k) -> p v k", v=98, k=4)
            nc.vector.tensor_add(out=h4, in0=h8[:, :, 0:4], in1=h8[:, :, 4:8])
            h2 = tn2[:, 1176:1372].rearrange("p (v k) -> p v k", v=98, k=2)
            nc.vector.tensor_add(out=h2, in0=h4[:, :, 0:2], in1=h4[:, :, 2:4])
            h1 = tn2[:, 1372:1470]
            nc.vector.tensor_add(out=h1, in0=h2[:, :, 0], in1=h2[:, :, 1])
            # s[t,g] = h1[t,0,g] + h1[t,1,g]
            s_all = tn2[:, 1470:1519]
            gm = h1.rearrange("p (t m g) -> p t m g", t=T, m=2, g=S)
            nc.vector.tensor_add(out=s_all.rearrange("p (t g) -> p t g",
                                                     t=T, g=S),
                                 in0=gm[:, :, 0], in1=gm[:, :, 1])
            # head: hm[c, tg] = s[tg] * w[c, tg]; o[c] = sum_tg hm
            hm = pr[:, 0:2 * GP].rearrange("p (c q) -> p c q", c=2, q=GP)
            nc.vector.tensor_mul(
                out=hm,
                in0=w_sb[:, :].rearrange("p (c q) -> p c q", c=2, q=GP),
                in1=s_all.unsqueeze(1).broadcast_to((P, 2, GP)))
            o_sb = big.tile([P, 2], f32)
            nc.vector.reduce_sum(out=o_sb[:, :].rearrange("p c -> p c"),
                                 in_=hm, axis=mybir.AxisListType.X)
            nc.sync.dma_start(out=o_d, in_=o_sb[:, :])

    return nc


def _get_nc():
    if "nc" not in _cache:
        _cache["nc"] = _build_nc()
        import concourse.mybir as mybir
        _split_multiwait(_cache["nc"], mybir)
    return _cache["nc"]


def kernel(h0, cw0, mw0, cw1, mw1,
           msg0_r0, msg0_r1, msg0_r2,
           msg1_r0, msg1_r1, msg1_r2,
           w_pred, b_pred):
    from concourse.bass_utils import run_bass_kernel_spmd

    nc = _get_nc()

    def pad_mat(m):
        out = np.zeros((NP, 256), np.float32)
        out[:N] = np.asarray(m, np.float32).reshape(N, 256)
        return out.reshape(CORES, T, 128, S, 16, 16)  # [c,t,p,g,j,k]

    A5 = pad_mat(cw0[0])
    B5 = pad_mat(mw0[0])
    # AB: [c,t,p, m,g,j,k] -> (c,t,p,3584)
    AB = np.ascontiguousarray(
        np.stack([A5, B5], axis=3).reshape(CORES, T, 128, 3584)).astype(BF16)

    D5 = pad_mat(cw1[0])
    E5 = pad_mat(mw1[0])
    # DE: j innermost for the fold tree: [c,t,p, m,g,k,j] -> (c,t,p,3584)
    DE = np.ascontiguousarray(
        np.stack([D5.transpose(0, 1, 2, 3, 5, 4),
                  E5.transpose(0, 1, 2, 3, 5, 4)], axis=3)
        .reshape(CORES, T, 128, 3584)).astype(BF16)

    def pad_vec(v):
        out = np.zeros((NP, 16), np.float32)
        out[:N] = np.asarray(v, np.float32).reshape(N, 16)
        return out.reshape(CORES, T, 128, S, 16)

    X = pad_vec(np.asarray(h0, np.float32)[..., 0])
    M0 = pad_vec(np.asarray(msg0_r0, np.float32)[..., 0])
    XM = np.ascontiguousarray(
        np.stack([X, M0], axis=3).reshape(CORES, T, 128, 224)
        .transpose(0, 2, 1, 3).reshape(CORES, 128, T * 224)).astype(BF16)
    M1 = np.ascontiguousarray(
        pad_vec(np.asarray(msg1_r0, np.float32)[..., 0])
        .reshape(CORES, T, 128, 112)
        .transpose(0, 2, 1, 3).reshape(CORES, 128, T * 112)).astype(BF16)

    wp = np.zeros((2, NP), np.float32)
    wp[:, :N] = np.asarray(w_pred, np.float32)
    W = np.ascontiguousarray(
        wp.reshape(2, CORES, T, 128, S).transpose(1, 3, 0, 2, 4)
        .reshape(CORES, 128, 2 * GP)).astype(BF16)

    in_maps = [
        {"ab": AB[i], "de": DE[i], "xm": XM[i], "m1": M1[i], "w": W[i]}
        for i in range(CORES)
    ]
    res = run_bass_kernel_spmd(nc, in_maps, list(range(CORES)), trace=TRACE)
    _cache["last_res"] = res
    partial = np.zeros(2, np.float64)
    for i in range(CORES):
        partial += res.results[i]["o"].astype(np.float64).sum(axis=0)
    out = (partial + np.asarray(b_pred, np.float64)).astype(np.float32)
    return out.reshape(1, 2)
